# revision 5
# baseline (speedup 1.0000x reference)
"""DistSageConv forward on 8 Trainium2 NeuronCores — pair-gather design.

Per core (partition p, half-of-segments h): kept edges + self rows are
"instances" (src row, seg block, loc = seg%128, type e/s). x is split
into two 50K-row halves; per half the host builds a permuted copy B of
the half's rows (plus a few duplicated multi-use rows) such that pairs
of same-block instances sit at adjacent (even, odd) positions. One
dma_gather element then covers TWO instances (512B), halving SWDGE
descriptor count — Q7 descriptor generation is the machine bottleneck.
Unpaired instances gather single rows through per-lane strided views of
the same B (elem_step=2 rows).

Per block b: one DVE tensor_scalar is_equal per (tile, lane, type) with
any label builds Sel[row, seg] from a per-partition loc column (scalar
operand — holds no streamed second SBUF read, so Q7's descriptor writes
are not locked out of the shared SBUF port pair), and one PE matmul
accumulates psum[din, seg] += lhsT.T @ Sel. Edge MMs alternate two PSUM
banks; self MMs use a third. zT = W2@(ngA+ngB) + W1@selfT + bias via a
linear chain, written as columns straight to DRAM; the host transposes
and reindexes (pure index work).
"""
import os
import numpy as np

import concourse.bass as bass
import concourse.bacc as bacc
import concourse.mybir as mybir
from concourse.tile import TileContext

F32 = mybir.dt.float32
BF16 = mybir.dt.float16
I16 = mybir.dt.int16
BF16_NP = np.float16

NCORES = 8
LAST_EXEC_NS = None
SEG_BLK = 128
NHALF = 2
B_CAP = 65024          # rows per permuted half-copy (pair elems <= 32512)
WIN_P = 1024           # pair elems per gather window
WIN_S = 1024           # single elems per gather window
RING = 3
PELEM = 256            # bf16 values per pair elem (2 rows)

import concourse.tile_sem_assignment as _tsa

if not getattr(_tsa, "_queue_lane_patch", False):
    _orig_assign_tick = _tsa.TileClockTick._assign_tick

    def _assign_tick_queue_aware(self, inst):
        if (
            isinstance(inst, _tsa.DMAInst)
            and inst.engine == mybir.EngineType.Pool
        ):
            self.next_sw_dma_idx = getattr(inst, "queue_num", 0) or 0
        return _orig_assign_tick(self, inst)

    _tsa.TileClockTick._assign_tick = _assign_tick_queue_aware
    _tsa._queue_lane_patch = True

# stream descriptors: (name, elem_size bf16, nlanes)
STREAMS = (("p", PELEM, 2), ("s", PELEM, 2))


def _wrap16(flat):
    n = len(flat)
    w = flat.reshape(n // 16, 16).T
    return np.tile(w, (8, 1))


def _prep_core(es, ed, sid, oid, half, ndst, nsrc):
    uniq = np.unique(oid)
    U = uniq[half::2]
    nu = len(U)
    seg_of_dst = np.full(ndst, -1, np.int32)
    seg_of_dst[U] = np.arange(nu, dtype=np.int32)

    seg_all = seg_of_dst[ed]
    keep = seg_all >= 0
    es_k = es[keep].astype(np.int64)
    seg_k = seg_all[keep].astype(np.int64)

    self_src = sid[U].astype(np.int64)
    s_seg = np.arange(nu, dtype=np.int64)

    src = np.concatenate([es_k, self_src])
    seg = np.concatenate([seg_k, s_seg])
    typ = np.concatenate([np.zeros(len(es_k), np.int8),
                          np.ones(nu, np.int8)])
    blk = seg // SEG_BLK
    loc = (seg % SEG_BLK).astype(np.float32)

    seg_out = seg_of_dst[oid]
    mine = seg_out >= 0
    rows = np.nonzero(mine)[0]
    oseg = seg_out[mine].astype(np.int64)
    return dict(nu=nu, src=src, blk=blk, loc=loc, typ=typ,
                rows=rows, oseg=oseg)


def _pair_half(src_l, blk, loc, typ, nrows):
    """Greedy pairing for one (core, half): pair instances within
    (blk, typ) groups subject to one B-copy per claimed instance."""
    n = len(src_l)
    order = np.lexsort((typ, blk))
    src_o, blk_o, loc_o, typ_o = (src_l[order], blk[order], loc[order],
                                  typ[order])

    u = np.bincount(src_o, minlength=nrows)
    spare = B_CAP - nrows
    extra = np.zeros(nrows, np.int64)
    if spare > 0 and n:
        cand = np.argsort(-u, kind="stable")
        want = np.maximum(u[cand] - 1, 0)
        cw = np.cumsum(want)
        k = int(np.searchsorted(cw, spare, side="right"))
        extra[cand[:k]] = want[:k]
        if k < nrows and want[k] > 0:
            extra[cand[k]] = spare - (cw[k - 1] if k else 0)
    slots = 1 + extra

    claimed = np.zeros(n, bool)
    rem = slots.copy()
    for i in range(n):
        s = src_o[i]
        if rem[s] > 0:
            rem[s] -= 1
            claimed[i] = True

    key = blk_o * 2 + typ_o
    B_rows = []
    inst_elem = np.empty(n, np.int64)
    inst_lane = np.zeros(n, np.int8)
    is_pair = np.zeros(n, bool)
    pos_of = np.full(nrows, -1, np.int64)
    bounds = np.searchsorted(key, np.arange(key.max() + 2 if n else 1))
    pend = []
    for g in range(len(bounds) - 1):
        idxs = np.arange(bounds[g], bounds[g + 1])
        cl = idxs[claimed[idxs]]
        npair = len(cl) // 2
        for k in range(npair):
            ia, ib = cl[2 * k], cl[2 * k + 1]
            q = len(B_rows)
            sa, sb = src_o[ia], src_o[ib]
            B_rows.append(sa); B_rows.append(sb)
            if pos_of[sa] < 0: pos_of[sa] = q
            if pos_of[sb] < 0: pos_of[sb] = q + 1
            inst_elem[ia] = q // 2; inst_lane[ia] = 0
            inst_elem[ib] = q // 2; inst_lane[ib] = 1
            is_pair[ia] = is_pair[ib] = True
        for i in idxs:
            if not is_pair[i]:
                pend.append(i)
    placed = np.zeros(nrows, bool)
    if B_rows:
        placed[np.array(B_rows, np.int64)] = True
    for s in np.nonzero(~placed)[0]:
        pos_of[s] = len(B_rows)
        B_rows.append(s)
    if len(B_rows) % 2:
        B_rows.append(0)
    for i in pend:
        q = pos_of[src_o[i]]
        inst_elem[i] = q // 2
        inst_lane[i] = q % 2
    B_rows = np.array(B_rows, np.int64)
    assert len(B_rows) <= B_CAP
    return dict(B=B_rows, elem=inst_elem, lane=inst_lane, blk=blk_o,
                loc=loc_o, typ=typ_o, is_pair=is_pair)


def _stream_members(pi, sname):
    """Instance mask for stream sname."""
    if sname == "p":
        return pi["is_pair"]
    return ~pi["is_pair"]


def _stream_layout(pinfos, sname, nb, win):
    """Static slab sizes per (blk, typ) for one (half, stream) = max over
    cores, 16-aligned so tiles are mostly type-pure."""
    nidx = np.zeros((nb, 2), np.int64)
    per_core = []
    for pi in pinfos:
        m = _stream_members(pi, sname)
        cnt = np.zeros((nb, 2), np.int64)
        core = []
        for b in range(nb):
            row = []
            for t in range(2):
                mb = m & (pi["blk"] == b) & (pi["typ"] == t)
                if sname == "p":
                    elems = np.unique(pi["elem"][mb])
                else:
                    elems = pi["elem"][mb]
                row.append(elems)
                cnt[b, t] = len(elems)
            core.append(row)
        per_core.append(core)
        nidx = np.maximum(nidx, cnt)
    nidx = ((nidx + 15) // 16) * 16
    if sname == "p":
        nidx[:, 0] = np.maximum(nidx[:, 0], 16)
        nidx[:, 1] = np.maximum(nidx[:, 1], 16)
    soff = np.zeros((nb, 2), np.int64)
    soff_blk = np.zeros(nb + 1, np.int64)
    acc = 0
    for b in range(nb):
        for t in range(2):
            soff[b, t] = acc
            acc += nidx[b, t]
        soff_blk[b + 1] = acc
    L = acc
    wins = []
    r = 0
    while r < L:
        n = min(win, L - r)
        wins.append((r, n))
        r += n
    return dict(nidx=nidx, soff=soff, soff_blk=soff_blk, wins=wins,
                per_core=per_core)


def _stream_flats(pi, lay, sname, nb, nlanes):
    L = int(lay["soff_blk"][nb])
    flat = np.zeros(L, np.int16)
    lab = np.full((L, nlanes, 2), -1.0, np.float32)
    m_all = _stream_members(pi, sname)
    for b in range(nb):
        for t in range(2):
            base = int(lay["soff"][b, t])
            mb = m_all & (pi["blk"] == b) & (pi["typ"] == t)
            if sname == "p":
                pe = np.unique(pi["elem"][mb])
                pos_of = {}
                for k, e in enumerate(pe):
                    flat[base + k] = e
                    pos_of[int(e)] = base + k
                for i in np.nonzero(mb)[0]:
                    p = pos_of[int(pi["elem"][i])]
                    lab[p, pi["lane"][i], t] = pi["loc"][i]
            else:
                si = np.nonzero(mb)[0]
                for k, i in enumerate(si):
                    flat[base + k] = pi["elem"][i]
                    lab[base + k, pi["lane"][i], t] = pi["loc"][i]
    return flat, lab


def _work_layout(lay, labs, nb, nlanes):
    work = [[] for _ in range(nb)]
    segcol = {}
    ncols = 0
    soff = lay["soff_blk"]
    for b in range(nb):
        r0, r1 = int(soff[b]), int(soff[b + 1])
        if r1 == r0:
            continue
        for j in range(r0 // 128, (r1 - 1) // 128 + 1):
            lo, hi = max(r0, j * 128), min(r1, (j + 1) * 128)
            for lane in range(nlanes):
                for typ in range(2):
                    if any((lab[lo:hi, lane, typ] >= 0).any() for lab in labs):
                        work[b].append((j, lane, typ))
                        segcol[(b, j, lane, typ)] = ncols
                        ncols += 1
    return work, segcol, ncols


def _core_segs(lab, lay, work, segcol, nb, ncols, colbase, segs):
    soff = lay["soff_blk"]
    for b in range(nb):
        r0, r1 = int(soff[b]), int(soff[b + 1])
        for (j, lane, typ) in work[b]:
            col = colbase + segcol[(b, j, lane, typ)]
            lo, hi = max(r0, j * 128), min(r1, (j + 1) * 128)
            segs[lo - j * 128 : hi - j * 128, col] = lab[lo:hi, lane, typ]


def _build_program(din, dout, nb, meta):
    """meta[(h, sname)] = dict(lay, work, segcol, ncols, colbase, goff,
    elem, nlanes, win)."""
    nc = bacc.Bacc(num_swdge_queues=4)
    ncols_tot = max(m["colbase"] + m["ncols"] for m in meta.values())
    gcols = max(sum(n // 16 for (_, n) in m["lay"]["wins"])
                for m in [None]) if False else 0
    gcols = 0
    for m in meta.values():
        gcols = max(gcols, m["goff"][-1] + m["lay"]["wins"][-1][1] // 16) \
            if m["lay"]["wins"] else gcols

    b_d = [nc.dram_tensor(f"bh{h}", [B_CAP // 2, PELEM], BF16,
                          kind="ExternalInput") for h in range(NHALF)]
    gidx_d = nc.dram_tensor("gidx", [128, max(gcols, 1)], I16,
                            kind="ExternalInput")
    segs_d = nc.dram_tensor("segs", [128, max(ncols_tot, 1)], F32,
                            kind="ExternalInput")
    w1t_d = nc.dram_tensor("w1t", [din, dout], BF16, kind="ExternalInput")
    w2t_d = nc.dram_tensor("w2t", [din, dout], BF16, kind="ExternalInput")
    bias_d = nc.dram_tensor("bias", [dout, 1], F32, kind="ExternalInput")
    iota_d = nc.dram_tensor("iota", [128, SEG_BLK], BF16, kind="ExternalInput")
    zt_d = nc.dram_tensor("zt", [dout, nb * SEG_BLK], F32,
                          kind="ExternalOutput")

    with TileContext(nc) as tc:
        with (
            tc.tile_pool(name="const", bufs=1) as cpool,
            tc.tile_pool(name="work", bufs=3) as wpool,
            tc.tile_pool(name="psE0", bufs=2, space="PSUM") as psE0,
            tc.tile_pool(name="psE1", bufs=2, space="PSUM") as psE1,
            tc.tile_pool(name="psS", bufs=2, space="PSUM") as psS,
            tc.tile_pool(name="psC", bufs=2, space="PSUM") as psC,
        ):
            gidx_sb = cpool.tile([128, max(gcols, 1)], I16)
            segs_sb = cpool.tile([128, max(ncols_tot, 1)], F32)
            w1t_sb = cpool.tile([din, dout], BF16)
            w2t_sb = cpool.tile([din, dout], BF16)
            bias_sb = cpool.tile([dout, 1], F32)
            iota_sb = cpool.tile([128, SEG_BLK], BF16)
            for sb_t, d_t in [(gidx_sb, gidx_d), (segs_sb, segs_d),
                              (w1t_sb, w1t_d), (w2t_sb, w2t_d),
                              (bias_sb, bias_d), (iota_sb, iota_d)]:
                nc.sync.dma_start(out=sb_t[:], in_=d_t[:])

            rings = {}
            for (h, sname), m in meta.items():
                wt = m["win"] // 128
                rings[(h, sname)] = [
                    cpool.tile([128, wt * m["elem"]], BF16,
                               tag=f"g{h}{sname}{r}", name=f"g{h}{sname}{r}")
                    for r in range(RING)]
                for r in range(RING):
                    wins = m["lay"]["wins"]
                    first = wins[r][1] if r < len(wins) else 0
                    if first < m["win"]:
                        nc.vector.memset(rings[(h, sname)][r][:], 0.0)

            issued = {k: 0 for k in meta}

            def issue(h, sname, wmax):
                m = meta[(h, sname)]
                while issued[(h, sname)] <= wmax:
                    w = issued[(h, sname)]
                    r0, n = m["lay"]["wins"][w]
                    nt = (n + 127) // 128
                    g = rings[(h, sname)][w % RING]
                    elem = m["elem"]
                    nc.gpsimd.dma_gather(
                        out_ap=g[:, : nt * elem].rearrange(
                            "p (t d) -> p t d", d=elem),
                        in_ap=b_d[h][:, :],
                        idxs_ap=gidx_sb[:, m["goff"][w] : m["goff"][w] + n // 16],
                        num_idxs=n, num_idxs_reg=n, elem_size=elem,
                        queue_num=(h * 2 + (0 if sname == "p" else 1)),
                    )
                    issued[(h, sname)] += 1

            for b in range(nb):
                for (h, sname), m in meta.items():
                    if m["work"][b]:
                        jmax = max(j for (j, _, _) in m["work"][b])
                        issue(h, sname, jmax // (m["win"] // 128))

                ngA = psE0.tile([din, SEG_BLK], F32, space="PSUM")
                ngB = psE1.tile([din, SEG_BLK], F32, space="PSUM")
                selfT = psS.tile([din, SEG_BLK], F32, space="PSUM")
                e_cnt = sum(1 for m in meta.values()
                            for (j, l, t) in m["work"][b] if t == 0)
                s_cnt = sum(1 for m in meta.values()
                            for (j, l, t) in m["work"][b] if t == 1)
                use_B = e_cnt > 1
                nA = (e_cnt + 1) // 2 if use_B else e_cnt
                nBc = e_cnt // 2
                ei = si = 0
                for (h, sname), m in meta.items():
                    wt = m["win"] // 128
                    for (j, lane, typ) in m["work"][b]:
                        col = m["colbase"] + m["segcol"][(b, j, lane, typ)]
                        sel = wpool.tile([128, SEG_BLK], BF16, tag="sel",
                                         bufs=8, name="sel")
                        nc.vector.tensor_scalar(
                            out=sel[:], in0=iota_sb[:],
                            scalar1=segs_sb[:, col : col + 1],
                            scalar2=None,
                            op0=mybir.AluOpType.is_equal,
                        )
                        g = rings[(h, sname)][(j // wt) % RING]
                        jc = j % wt
                        off = jc * m["elem"] + lane * din
                        lhsT = g[:, off : off + din]
                        if typ == 0:
                            if use_B and (ei % 2 == 1):
                                ps, k, n_mm = ngB, ei // 2, nBc
                            else:
                                ps, k, n_mm = ngA, (ei // 2 if use_B else ei), nA
                            nc.tensor.matmul(out=ps[:], lhsT=lhsT, rhs=sel[:],
                                             start=(k == 0),
                                             stop=(k == n_mm - 1))
                            ei += 1
                        else:
                            nc.tensor.matmul(out=selfT[:], lhsT=lhsT,
                                             rhs=sel[:], start=(si == 0),
                                             stop=(si == s_cnt - 1))
                            si += 1

                ngA_sb = wpool.tile([din, SEG_BLK], BF16, tag="ngA")
                nc.scalar.copy(out=ngA_sb[:], in_=ngA[:])
                if use_B:
                    ngB_sb = wpool.tile([din, SEG_BLK], BF16, tag="ngB")
                    nc.scalar.copy(out=ngB_sb[:], in_=ngB[:])
                selfT_sb = wpool.tile([din, SEG_BLK], BF16, tag="selfT")
                nc.scalar.copy(out=selfT_sb[:], in_=selfT[:])

                zT = psC.tile([dout, SEG_BLK], F32, space="PSUM")
                nc.tensor.matmul(out=zT[:], lhsT=w2t_sb[:], rhs=ngA_sb[:],
                                 start=True, stop=False)
                if use_B:
                    nc.tensor.matmul(out=zT[:], lhsT=w2t_sb[:], rhs=ngB_sb[:],
                                     start=False, stop=False)
                nc.tensor.matmul(out=zT[:], lhsT=w1t_sb[:], rhs=selfT_sb[:],
                                 start=False, stop=True)
                zT_sb = wpool.tile([dout, SEG_BLK], F32, tag="zT")
                nc.scalar.activation(out=zT_sb[:], in_=zT[:],
                                     func=mybir.ActivationFunctionType.Identity,
                                     bias=bias_sb[:])
                nc.sync.dma_start(
                    out=zt_d[:, b * SEG_BLK : (b + 1) * SEG_BLK], in_=zT_sb[:])
    nc.finalize()
    return nc


def _host_prepare(x, W, b, edge_src, edge_dst, self_ids, owned_ids):
    P, nsrc, din = x.shape
    ndst = max(int(edge_dst.max()), int(owned_ids.max())) + 1
    dout = W.shape[0]
    half_rows = nsrc // NHALF

    preps = []
    for c in range(NCORES):
        p, h = c // 2, c % 2
        preps.append(_prep_core(edge_src[p], edge_dst[p], self_ids[p],
                                owned_ids[p], h, ndst, nsrc))
    nb = max((pr["nu"] + SEG_BLK - 1) // SEG_BLK for pr in preps)

    pinfos = [[None] * NHALF for _ in range(NCORES)]
    for c in range(NCORES):
        pr = preps[c]
        hi = pr["src"] // half_rows
        for h in range(NHALF):
            m = hi == h
            pinfos[c][h] = _pair_half(pr["src"][m] - h * half_rows,
                                      pr["blk"][m], pr["loc"][m],
                                      pr["typ"][m], half_rows)

    meta = {}
    colbase = 0
    goff0 = 0
    flats_all = {}
    labs_all = {}
    for h in range(NHALF):
        for (sname, elem, nlanes) in STREAMS:
            win = WIN_P if sname == "p" else WIN_S
            lay = _stream_layout([pinfos[c][h] for c in range(NCORES)],
                                 sname, nb, win)
            labs = []
            flats = []
            for c in range(NCORES):
                fi, lab = _stream_flats(pinfos[c][h], lay, sname, nb, nlanes)
                flats.append(fi)
                labs.append(lab)
            work, segcol, ncols = _work_layout(lay, labs, nb, nlanes)
            goff = []
            off = goff0
            for (r0, n) in lay["wins"]:
                goff.append(off)
                off += n // 16
            meta[(h, sname)] = dict(lay=lay, work=work, segcol=segcol,
                                    ncols=ncols, colbase=colbase, goff=goff,
                                    elem=elem, nlanes=nlanes, win=win)
            flats_all[(h, sname)] = flats
            labs_all[(h, sname)] = labs
            colbase += ncols
            goff0 = off

    ncols_tot = colbase
    in_maps = []
    for c in range(NCORES):
        p = c // 2
        segs = np.full((128, max(ncols_tot, 1)), -1.0, np.float32)
        gparts = []
        bhs = {}
        for h in range(NHALF):
            B = pinfos[c][h]["B"]
            xb = np.zeros((B_CAP, din), BF16_NP)
            xh = x[p, h * half_rows : (h + 1) * half_rows].astype(BF16_NP)
            xb[: len(B)] = xh[B]
            bhs[f"bh{h}"] = np.ascontiguousarray(xb.reshape(B_CAP // 2, PELEM))
        for (h, sname), m in meta.items():
            _core_segs(labs_all[(h, sname)][c], m["lay"], m["work"],
                       m["segcol"], nb, m["ncols"], m["colbase"], segs)
            fi = flats_all[(h, sname)][c]
            for (r0, n) in m["lay"]["wins"]:
                gparts.append(_wrap16(fi[r0 : r0 + n]))
        gidx = (np.concatenate(gparts, axis=1) if gparts
                else np.zeros((128, 1), np.int16))
        in_maps.append(dict(
            gidx=np.ascontiguousarray(gidx), segs=np.ascontiguousarray(segs),
            w1t=np.ascontiguousarray(W[:, :din].T).astype(BF16_NP),
            w2t=np.ascontiguousarray(W[:, din:].T).astype(BF16_NP),
            bias=np.ascontiguousarray(b[:, None]).astype(np.float32),
            iota=np.tile(np.arange(SEG_BLK, dtype=np.float32),
                         (128, 1)).astype(BF16_NP),
            **bhs,
        ))
    return preps, nb, meta, flats_all, in_maps, din, dout


def emulate_core(in_map, meta, flats_core, nb, din, dout):
    zt = np.zeros((dout, nb * SEG_BLK), np.float32)
    w1t = in_map["w1t"].astype(np.float32)
    w2t = in_map["w2t"].astype(np.float32)
    bias = in_map["bias"][:, 0]
    iota = np.arange(SEG_BLK, dtype=np.float32)
    segs = in_map["segs"]
    for b in range(nb):
        ng = np.zeros((din, SEG_BLK), np.float32)
        sf = np.zeros((din, SEG_BLK), np.float32)
        for (h, sname), m in meta.items():
            Bh = in_map[f"bh{h}"].reshape(-1, din).astype(np.float32)
            flat = flats_core[(h, sname)]
            L = int(m["lay"]["soff_blk"][nb])
            for (j, lane, typ) in m["work"][b]:
                col = m["colbase"] + m["segcol"][(b, j, lane, typ)]
                loc = segs[:, col]
                sel = (iota[None, :] == loc[:, None]).astype(np.float32)
                idxs = np.zeros(128, np.int64)
                lo, hi = j * 128, min((j + 1) * 128, L)
                idxs[: hi - lo] = flat[lo:hi]
                rows = Bh[idxs * 2 + lane]
                contrib = rows.T @ sel
                if typ == 0:
                    ng += contrib
                else:
                    sf += contrib
        zt[:, b * SEG_BLK : (b + 1) * SEG_BLK] = (
            w2t.T @ ng + w1t.T @ sf + bias[:, None])
    return zt


def kernel(x, W, b, edge_src, edge_dst, self_ids, owned_ids):
    x = np.asarray(x); W = np.asarray(W); b = np.asarray(b)
    edge_src = np.asarray(edge_src); edge_dst = np.asarray(edge_dst)
    self_ids = np.asarray(self_ids); owned_ids = np.asarray(owned_ids)

    P, nsrc, din = x.shape
    nown = owned_ids.shape[1]
    dout = W.shape[0]

    preps, nb, meta, flats_all, in_maps, din, dout = _host_prepare(
        x, W, b, edge_src, edge_dst, self_ids, owned_ids)

    nc = _build_program(din, dout, nb, meta)

    if os.environ.get("BASS_KERNEL_SIM"):
        from concourse.bass_interp import MultiCoreSim
        sim = MultiCoreSim(nc, NCORES)
        for c in range(NCORES):
            for k, v in in_maps[c].items():
                sim.cores[c].tensor(k)[:] = v
        sim.simulate()
        results = [{"zt": sim.cores[c].tensor("zt").copy()}
                   for c in range(NCORES)]
    else:
        from concourse.bass_utils import run_bass_kernel_spmd
        trace = bool(os.environ.get("BASS_KERNEL_TRACE"))
        if trace:
            import sys, types
            if "antenv.axon_hooks" not in sys.modules:
                mod = types.ModuleType("antenv.axon_hooks")
                mod._hook = None
                mod.set_axon_ntff_profile_hook = lambda h: setattr(mod, "_hook", h)
                mod.get_axon_ntff_profile_hook = lambda: mod._hook
                sys.modules["antenv.axon_hooks"] = mod
                import antenv
                antenv.axon_hooks = mod
                from trn_agent_boot.trn_boot import _ntff_profile_via_ctypes
                mod.set_axon_ntff_profile_hook(
                    _ntff_profile_via_ctypes("/opt/axon/libaxon_pjrt.so"))
        res = run_bass_kernel_spmd(nc, in_maps, list(range(NCORES)),
                                   trace=trace, trace_cores=[0] if trace else None,
                                   tmpdir=os.environ.get("BASS_KERNEL_TRACE_DIR"))
        results = res.results
        global LAST_EXEC_NS
        LAST_EXEC_NS = res.exec_time_ns

    out = np.empty((P, nown, dout), np.float32)
    for c in range(NCORES):
        p = c // 2
        pr = preps[c]
        out[p, pr["rows"]] = results[c]["zt"][:, pr["oseg"]].T
    return out


# revision 6
# speedup vs baseline: 1.1945x; 1.1945x over previous
"""DistSageConv forward on 8 Trainium2 NeuronCores (Bass/Tile).

Math per graph partition p (of 4):
    ng  = segment_sum(x[edge_src], edge_dst, NDST)          # neighbor agg
    out = x[self_ids[owned_ids]] @ W1.T + ng[owned_ids] @ W2.T + b
          (W1 = W[:, :DIN], W2 = W[:, DIN:])

Only dst nodes appearing in owned_ids matter, so edges to non-owned dst are
dropped on the host (~60%). Each partition is split across 2 cores by
interleaving its unique owned dst ids ("segments"); segments are processed
in blocks of 128.

Edges and self rows are laid out host-side as four continuous per-src-chunk
streams (chunking keeps dma_gather's int16 indices in range), sliced into
1024-row gather windows on four SWDGE queues. Per block the device builds
one-hot selection tiles SelT[e, s] = (seg_local[e] == s) with one DVE
tensor_scalar is_equal per tile (the per-partition seg value is the scalar
operand, so the op holds no streamed second SBUF read — SWDGE descriptor
generation on the Q7 is not locked out of the shared SBUF port pair) and
accumulates ngT[din, seg] += xs_tile.T @ SelT on the PE into PSUM (bf16
data, fp32 accumulate), alternating between two PSUM banks to avoid the
same-address accumulate serialization; self rows flow through the same
machinery into a third PSUM. Then zT = W2T.T@ngT + W1T.T@selfT (+bias on
ACT) and zT columns are written straight to DRAM; the host transposes and
gathers the owned rows from zT (pure index work).
"""
import os
import numpy as np

import concourse.bass as bass
import concourse.bacc as bacc
import concourse.mybir as mybir
from concourse.tile import TileContext

F32 = mybir.dt.float32
BF16 = mybir.dt.float16
I32 = mybir.dt.int32
I16 = mybir.dt.int16
BF16_NP = np.float16

NCORES = 8
LAST_EXEC_NS = None
SEG_BLK = 128
# src chunk boundaries as fractions of NSRC (chunk sizes must stay <32768
# for int16 gather indices)
CHUNK_FRACS = (0.0, 0.25, 0.5, 0.75, 1.0)
GATHER_WIN = 1024
RING = 6
RING_S = 3

# Tile's sem assignment round-robins SWDGE DMA insts across DMASW lanes
# with no regard for queue_num, but each DMA semaphore may only be updated
# from one SWDGE queue. Pin lane = queue_num so multi-queue gathers are
# legal.
import concourse.tile_sem_assignment as _tsa

if not getattr(_tsa, "_queue_lane_patch", False):
    _orig_assign_tick = _tsa.TileClockTick._assign_tick

    def _assign_tick_queue_aware(self, inst):
        if (
            isinstance(inst, _tsa.DMAInst)
            and inst.engine == mybir.EngineType.Pool
        ):
            self.next_sw_dma_idx = getattr(inst, "queue_num", 0) or 0
        return _orig_assign_tick(self, inst)

    _tsa.TileClockTick._assign_tick = _assign_tick_queue_aware
    _tsa._queue_lane_patch = True


def _wrap16(flat):
    """dma_gather index layout: idx i -> [i % 16, i // 16], replicated to
    all 8 groups of 16 partitions. len(flat) must be a multiple of 16."""
    n = len(flat)
    w = flat.reshape(n // 16, 16).T
    return np.tile(w, (8, 1))


def _chunk_cuts(nsrc):
    cuts = [int(round(f * nsrc)) for f in CHUNK_FRACS]
    cuts[0], cuts[-1] = 0, nsrc
    for a, b in zip(cuts, cuts[1:]):
        assert 0 < b - a < 32768
    return np.array(cuts, np.int64)


def _prep_core(es, ed, sid, oid, half, ndst, cuts):
    """Host-side index prep for one core (partition p, half h)."""
    uniq = np.unique(oid)
    U = uniq[half::2]
    nu = len(U)
    seg_of_dst = np.full(ndst, -1, np.int32)
    seg_of_dst[U] = np.arange(nu, dtype=np.int32)

    seg_all = seg_of_dst[ed]
    keep = seg_all >= 0
    es_k = es[keep].astype(np.int64)
    seg_k = seg_all[keep].astype(np.int64)
    blk = seg_k // SEG_BLK
    loc = (seg_k % SEG_BLK).astype(np.float32)
    ch = np.searchsorted(cuts, es_k, side="right") - 1

    order = np.lexsort((ch, blk))
    es_o = (es_k - cuts[ch])[order]
    loc_o = loc[order]
    key_o = (blk * 4 + ch)[order]

    self_src = sid[U]
    s_seg = np.arange(nu, dtype=np.int64)
    s_blk = s_seg // SEG_BLK
    s_loc = (s_seg % SEG_BLK).astype(np.float32)
    s_ch = np.searchsorted(cuts, self_src, side="right") - 1
    s_order = np.lexsort((s_ch, s_blk))
    s_es = (self_src - cuts[s_ch])[s_order]
    s_loc = s_loc[s_order]
    s_key = (s_blk * 4 + s_ch)[s_order]
    seg_out = seg_of_dst[oid]
    mine = seg_out >= 0
    rows = np.nonzero(mine)[0]
    oseg = seg_out[mine].astype(np.int64)
    return dict(nu=nu, es=es_o, loc=loc_o, key=key_o,
                s_es=s_es, s_loc=s_loc, s_key=s_key,
                rows=rows, oseg=oseg)


def _slab_sizes(preps, nb, field):
    """Static per-(block, chunk) gather sizes: max count over cores,
    rounded up to 16 (dma_gather idx wrap granularity)."""
    nb4 = nb * 4
    gmax = np.zeros(nb4, np.int64)
    for pr in preps:
        cnt = np.bincount(pr[field], minlength=nb4)
        gmax = np.maximum(gmax, cnt)
    nidx = ((gmax + 15) // 16) * 16
    # make sure every block has at least one tile so its psum is written
    for b in range(nb):
        if nidx[b * 4 : (b + 1) * 4].sum() == 0:
            nidx[b * 4] = 16
    tiles = (nidx + 127) // 128
    return nidx.astype(int), tiles.astype(int)


def _stream_layout(nidx, tiles, nb):
    """Static per-chunk stream layout from slab sizes.

    Returns per chunk c: slab offsets soff_rows[c][b], gather window sizes
    wins[c] (list of (row_start, n_rows)), the per-block matmul worklist
    (c, tile_j) pairs, and the seg-column counter layout.
    """
    GW = GATHER_WIN
    soff_rows = np.zeros((4, nb + 1), np.int64)
    for c in range(4):
        for b in range(nb):
            soff_rows[c][b + 1] = soff_rows[c][b] + nidx[b * 4 + c]
    wins = []
    for c in range(4):
        L = int(soff_rows[c][nb])
        w = []
        r = 0
        while r < L:
            n = min(GW, L - r)
            w.append((r, n))
            r += n
        wins.append(w)
    # per-block worklist: (c, j) for tiles j intersecting block b
    work = [[] for _ in range(nb)]
    for c in range(4):
        for b in range(nb):
            r0, r1 = int(soff_rows[c][b]), int(soff_rows[c][b + 1])
            if r1 == r0:
                continue
            j0, j1 = r0 // 128, (r1 - 1) // 128
            for j in range(j0, j1 + 1):
                work[b].append((c, j))
    # seg column index for each (b, c, j), ordered by block then position
    segcol = {}
    ncols = 0
    for b in range(nb):
        for (c, j) in work[b]:
            segcol[(b, c, j)] = ncols
            ncols += 1
    return soff_rows, wins, work, segcol, ncols


def _flat_streams(key, es, loc, nidx, soff_rows, nb):
    nb4 = nb * 4
    starts = np.searchsorted(key, np.arange(nb4 + 1))
    ofs = np.arange(len(key)) - starts[key]
    flat_idx = [np.zeros(int(soff_rows[c][nb]), np.int16) for c in range(4)]
    flat_seg = [np.full(int(soff_rows[c][nb]), -1.0, np.float32) for c in range(4)]
    for b in range(nb):
        for c in range(4):
            s = b * 4 + c
            sl = slice(starts[s], starts[s + 1])
            base = int(soff_rows[c][b])
            flat_idx[c][base + ofs[sl]] = es[sl].astype(np.int16)
            flat_seg[c][base + ofs[sl]] = loc[sl]
    return flat_idx, flat_seg


def _emit_stream(flat_idx, flat_seg, layout, nb, segs, colbase):
    soff_rows, wins, work, segcol, ncols = layout
    gparts = []
    for c in range(4):
        for (r0, n) in wins[c]:
            gparts.append(_wrap16(flat_idx[c][r0 : r0 + n]))
    for b in range(nb):
        for c in range(4):
            r0b, r1b = int(soff_rows[c][b]), int(soff_rows[c][b + 1])
            if r1b == r0b:
                continue
            for j in range(r0b // 128, (r1b - 1) // 128 + 1):
                col = colbase + segcol[(b, c, j)]
                t0 = j * 128
                lo, hi = max(r0b, t0), min(r1b, t0 + 128)
                segs[lo - t0 : hi - t0, col] = flat_seg[c][lo:hi]
    return gparts


def _build_streams(prep, nb, e_layout, s_layout, e_nidx, s_nidx):
    ncols_e, ncols_s = e_layout[4], s_layout[4]
    segs = np.full((128, max(ncols_e + ncols_s, 1)), -1.0, np.float32)
    fi, fs = _flat_streams(prep["key"], prep["es"], prep["loc"], e_nidx,
                           e_layout[0], nb)
    gparts = _emit_stream(fi, fs, e_layout, nb, segs, 0)
    fi2, fs2 = _flat_streams(prep["s_key"], prep["s_es"], prep["s_loc"],
                             s_nidx, s_layout[0], nb)
    gparts += _emit_stream(fi2, fs2, s_layout, nb, segs, ncols_e)
    gidx = (np.concatenate(gparts, axis=1) if gparts
            else np.zeros((128, 1), np.int16))
    return dict(gidx=np.ascontiguousarray(gidx),
                segs=np.ascontiguousarray(segs))


def _build_program(nsrc, din, dout, nb, cuts, e_layout, s_layout):
    nc = bacc.Bacc(num_swdge_queues=4)
    GW = GATHER_WIN
    WT = GW // 128
    e_soff, e_wins, e_work, e_segcol, e_ncols = e_layout
    s_soff, s_wins, s_work, s_segcol, s_ncols = s_layout
    ncols = e_ncols + s_ncols

    goff = {}
    off = 0
    for tag, wins in (("e", e_wins), ("s", s_wins)):
        for c in range(4):
            for w, (r0, n) in enumerate(wins[c]):
                goff[(tag, c, w)] = off
                off += n // 16
    gcols = max(off, 1)

    x_d = nc.dram_tensor("x", [nsrc, din], BF16, kind="ExternalInput")
    gidx_d = nc.dram_tensor("gidx", [128, gcols], I16, kind="ExternalInput")
    segs_d = nc.dram_tensor("segs", [128, max(ncols, 1)], F32, kind="ExternalInput")
    w1t_d = nc.dram_tensor("w1t", [din, dout], BF16, kind="ExternalInput")
    w2t_d = nc.dram_tensor("w2t", [din, dout], BF16, kind="ExternalInput")
    bias_d = nc.dram_tensor("bias", [dout, 1], F32, kind="ExternalInput")
    iota_d = nc.dram_tensor("iota", [128, SEG_BLK], BF16, kind="ExternalInput")

    zt_d = nc.dram_tensor("zt", [dout, nb * SEG_BLK], F32, kind="ExternalOutput")

    with TileContext(nc) as tc:
        with (
            tc.tile_pool(name="const", bufs=1) as cpool,
            tc.tile_pool(name="work", bufs=3) as wpool,
            tc.tile_pool(name="psE0", bufs=2, space="PSUM") as psE0,
            tc.tile_pool(name="psE1", bufs=2, space="PSUM") as psE1,
            tc.tile_pool(name="psS", bufs=2, space="PSUM") as psS,
            tc.tile_pool(name="psC", bufs=2, space="PSUM") as psC,
        ):
            gidx_sb = cpool.tile([128, gcols], I16)
            segs_sb = cpool.tile([128, max(ncols, 1)], F32)
            w1t_sb = cpool.tile([din, dout], BF16)
            w2t_sb = cpool.tile([din, dout], BF16)
            bias_sb = cpool.tile([dout, 1], F32)
            iota_sb = cpool.tile([128, SEG_BLK], BF16)
            for sb_t, d_t in [(gidx_sb, gidx_d), (segs_sb, segs_d),
                              (w1t_sb, w1t_d), (w2t_sb, w2t_d),
                              (bias_sb, bias_d), (iota_sb, iota_d)]:
                nc.sync.dma_start(out=sb_t[:], in_=d_t[:])

            # per-chunk rings of gather window buffers (edge + self), zeroed
            # only where the first window written to a slot is ragged
            ering = [[cpool.tile([128, WT * din], BF16, tag=f"er{c}_{r}",
                                 name=f"er{c}_{r}") for r in range(RING)]
                     for c in range(4)]
            sring = [[cpool.tile([128, WT * din], BF16, tag=f"sr{c}_{r}",
                                 name=f"sr{c}_{r}") for r in range(RING_S)]
                     for c in range(4)]
            for grp, nring, wins in ((ering, RING, e_wins),
                                     (sring, RING_S, s_wins)):
                for c in range(4):
                    nwin = len(wins[c])
                    for r in range(nring):
                        first = wins[c][r][1] if r < nwin else 0
                        if first < 128 * WT:
                            nc.vector.memset(grp[c][r][:], 0.0)

            e_issued = [0, 0, 0, 0]
            s_issued = [0, 0, 0, 0]

            def issue(tag, wins, ring_grp, nring, issued, c, wmax):
                while issued[c] <= wmax:
                    w = issued[c]
                    r0, n = wins[c][w]
                    nt = (n + 127) // 128
                    g = ring_grp[c][w % nring]
                    nc.gpsimd.dma_gather(
                        out_ap=g[:, : nt * din].rearrange("p (t d) -> p t d", d=din),
                        in_ap=x_d[int(cuts[c]) : int(cuts[c + 1]), :],
                        idxs_ap=gidx_sb[:, goff[(tag, c, w)] : goff[(tag, c, w)] + n // 16],
                        num_idxs=n, num_idxs_reg=n, elem_size=din,
                        queue_num=c,
                    )
                    issued[c] += 1

            def accum(ps_tiles, worklist, segcol, colbase, ring_grp, nring):
                """One-hot accumulate worklist tiles into len(ps_tiles)
                alternating PSUM tiles; returns #mms issued per psum."""
                n_mm = len(worklist)
                nps = len(ps_tiles)
                counts = [0] * nps
                order = [0] * n_mm
                for i in range(n_mm):
                    order[i] = i % nps
                    counts[i % nps] += 1
                seen = [0] * nps
                for i_mm, (c, j) in enumerate(worklist):
                    col = colbase + segcol[(b, c, j)]
                    sel = wpool.tile([128, SEG_BLK], BF16, tag="sel",
                                     bufs=6, name="sel")
                    nc.vector.tensor_scalar(
                        out=sel[:], in0=iota_sb[:],
                        scalar1=segs_sb[:, col : col + 1],
                        scalar2=None,
                        op0=mybir.AluOpType.is_equal,
                    )
                    buf = ring_grp[c][(j // WT) % nring]
                    bc = j % WT
                    p = order[i_mm]
                    nc.tensor.matmul(
                        out=ps_tiles[p][:],
                        lhsT=buf[:, bc * din : (bc + 1) * din],
                        rhs=sel[:],
                        start=(seen[p] == 0), stop=(seen[p] == counts[p] - 1),
                    )
                    seen[p] += 1
                return counts

            for b in range(nb):
                for c in range(4):
                    js = [j for (cc, j) in e_work[b] if cc == c]
                    if js:
                        issue("e", e_wins, ering, RING, e_issued, c, max(js) // WT)
                    sjs = [j for (cc, j) in s_work[b] if cc == c]
                    if sjs:
                        issue("s", s_wins, sring, RING_S, s_issued, c, max(sjs) // WT)

                ngA = psE0.tile([din, SEG_BLK], F32, space="PSUM")
                if len(e_work[b]) > 1:
                    ngB = psE1.tile([din, SEG_BLK], F32, space="PSUM")
                    accum([ngA, ngB], e_work[b], e_segcol, 0, ering, RING)
                else:
                    accum([ngA], e_work[b], e_segcol, 0, ering, RING)
                    ngB = None
                selfT = psS.tile([din, SEG_BLK], F32, space="PSUM")
                accum([selfT], s_work[b], s_segcol, e_ncols, sring, RING_S)

                ngA_sb = wpool.tile([din, SEG_BLK], BF16, tag="ngA")
                nc.scalar.copy(out=ngA_sb[:], in_=ngA[:])
                if ngB is not None:
                    ngB_sb = wpool.tile([din, SEG_BLK], BF16, tag="ngB")
                    nc.scalar.copy(out=ngB_sb[:], in_=ngB[:])
                else:
                    ngB_sb = None
                selfT_sb = wpool.tile([din, SEG_BLK], BF16, tag="selfT")
                nc.scalar.copy(out=selfT_sb[:], in_=selfT[:])

                zT = psC.tile([dout, SEG_BLK], F32, space="PSUM")
                nc.tensor.matmul(out=zT[:], lhsT=w2t_sb[:], rhs=ngA_sb[:],
                                 start=True, stop=False)
                if ngB_sb is not None:
                    nc.tensor.matmul(out=zT[:], lhsT=w2t_sb[:], rhs=ngB_sb[:],
                                     start=False, stop=False)
                nc.tensor.matmul(out=zT[:], lhsT=w1t_sb[:], rhs=selfT_sb[:],
                                 start=False, stop=True)
                zT_sb = wpool.tile([dout, SEG_BLK], F32, tag="zT")
                nc.scalar.activation(out=zT_sb[:], in_=zT[:],
                                     func=mybir.ActivationFunctionType.Identity,
                                     bias=bias_sb[:])
                nc.sync.dma_start(
                    out=zt_d[:, b * SEG_BLK : (b + 1) * SEG_BLK], in_=zT_sb[:])
    nc.finalize()
    return nc


def kernel(x, W, b, edge_src, edge_dst, self_ids, owned_ids):
    x = np.asarray(x); W = np.asarray(W); b = np.asarray(b)
    edge_src = np.asarray(edge_src); edge_dst = np.asarray(edge_dst)
    self_ids = np.asarray(self_ids); owned_ids = np.asarray(owned_ids)

    P, nsrc, din = x.shape
    ndst = max(int(edge_dst.max()), int(owned_ids.max())) + 1
    nown = owned_ids.shape[1]
    dout = W.shape[0]
    cuts = _chunk_cuts(nsrc)

    preps = []
    for c in range(NCORES):
        p, h = c // 2, c % 2
        preps.append(_prep_core(edge_src[p], edge_dst[p], self_ids[p],
                                owned_ids[p], h, ndst, cuts))

    nb = max((pr["nu"] + SEG_BLK - 1) // SEG_BLK for pr in preps)
    e_nidx, e_tiles = _slab_sizes(preps, nb, "key")
    s_nidx, s_tiles = _slab_sizes(preps, nb, "s_key")
    e_layout = _stream_layout(e_nidx, e_tiles, nb)
    s_layout = _stream_layout(s_nidx, s_tiles, nb)

    w1t = np.ascontiguousarray(W[:, :din].T).astype(BF16_NP)
    w2t = np.ascontiguousarray(W[:, din:].T).astype(BF16_NP)
    bias = np.ascontiguousarray(b[:, None]).astype(np.float32)
    iota = np.tile(np.arange(SEG_BLK, dtype=np.float32), (128, 1)).astype(BF16_NP)

    in_maps = []
    for c in range(NCORES):
        st = _build_streams(preps[c], nb, e_layout, s_layout,
                            e_nidx, s_nidx)
        in_maps.append(dict(
            x=np.ascontiguousarray(x[c // 2]).astype(BF16_NP),
            gidx=st["gidx"], segs=st["segs"],
            w1t=w1t, w2t=w2t, bias=bias,
            iota=np.ascontiguousarray(iota),
        ))

    nc = _build_program(nsrc, din, dout, nb, cuts, e_layout, s_layout)

    if os.environ.get("BASS_KERNEL_SIM"):
        from concourse.bass_interp import MultiCoreSim
        sim = MultiCoreSim(nc, NCORES)
        for c in range(NCORES):
            for k, v in in_maps[c].items():
                sim.cores[c].tensor(k)[:] = v
        sim.simulate()
        results = [{"zt": sim.cores[c].tensor("zt").copy()}
                   for c in range(NCORES)]
    else:
        from concourse.bass_utils import run_bass_kernel_spmd
        trace = bool(os.environ.get("BASS_KERNEL_TRACE"))
        if trace:
            import sys, types
            if "antenv.axon_hooks" not in sys.modules:
                mod = types.ModuleType("antenv.axon_hooks")
                mod._hook = None
                mod.set_axon_ntff_profile_hook = lambda h: setattr(mod, "_hook", h)
                mod.get_axon_ntff_profile_hook = lambda: mod._hook
                sys.modules["antenv.axon_hooks"] = mod
                import antenv
                antenv.axon_hooks = mod
                from trn_agent_boot.trn_boot import _ntff_profile_via_ctypes
                mod.set_axon_ntff_profile_hook(
                    _ntff_profile_via_ctypes("/opt/axon/libaxon_pjrt.so"))
        res = run_bass_kernel_spmd(nc, in_maps, list(range(NCORES)),
                                   trace=trace, trace_cores=[0] if trace else None,
                                   tmpdir=os.environ.get("BASS_KERNEL_TRACE_DIR"))
        results = res.results
        global LAST_EXEC_NS
        LAST_EXEC_NS = res.exec_time_ns

    out = np.empty((P, nown, dout), np.float32)
    for c in range(NCORES):
        p = c // 2
        pr = preps[c]
        out[p, pr["rows"]] = results[c]["zt"][:, pr["oseg"]].T
    return out
